# revision 1
# baseline (speedup 1.0000x reference)
"""AdversarialMorphingLoss — Trainium2 Bass kernel (8-core data parallel).

Full inputs arrive on the host; we shard the batch dim (B=4096) into 8
contiguous blocks of 512 rows, run one SPMD Bass program on all 8
NeuronCores, and each core returns the partial (un-normalized) sum of the
per-sample loss contribution over its 512 rows.  The host sums the 8
partials and divides by B.

Per-sample math (matching reference.py):
  scores_b = 100/S * sum_s inc_s * CONFIG_MULT[pid % 4]
  inc_s    = 0.6*(sz_s > 1400) + 0.4*(dly_s < 0.05)
           + 0.2*(|sz_s - sz_{s-1}| < 0.5) + 0.1*(dir_s != dir_{s-1})
  with sz[:, -1] -> min(sz[:, -1] + pad*1500, 1500), dly[:, -1] += delay_ms,
  and the s=0 "prev" being -1.0 (so the dir term contributes 0.1 at s=0 and
  the size-equality term contributes 0).

  c_b = (2/30)*relu(scores-15) + 0.5*(|dly_ms - TD[pid]| + |pad - TP[pid]|)
      + 0.3*(relu(dly_ms-20)/20 + relu(pad-0.3)) + 0.2*(conf - (scores<30))^2
  loss = mean_b c_b

On-device strategy (memory-bound: streams 96 MB of traces):
  * count (sz > 1400) over all S int32 cols with one ScalarE
    activation(Sign, bias=-1400.5, accum_out=...) per [128, 2048] tile
    (integers never hit the .5 threshold -> exact), then patch the last
    (float-modified) column with exact [128,4] is_gt ops.
  * count (dly < 0.05) the same way via Sign(0.05 - dly).
  * count consecutive-size equality / direction flips with one fused
    VectorE tensor_tensor_reduce(is_equal / not_equal, accum_out=...) per
    tile, again patching the last column separately.
  * everything per-sample afterwards runs on tiny [128, 4] tiles.
"""

import numpy as np
from contextlib import ExitStack

import concourse.bass as bass
import concourse.bacc as bacc
import concourse.mybir as mybir
from concourse import tile
from concourse.bass_utils import run_bass_kernel_spmd

B, S = 4096, 2048
N_CORES = 8
BC = B // N_CORES          # 512 rows per core
P = 128                    # SBUF partitions
NT = BC // P               # 4 tiles of 128 rows per core

F32 = mybir.dt.float32
I32 = mybir.dt.int32
ALU = mybir.AluOpType
ACTF = mybir.ActivationFunctionType

_NC_CACHE = None
LAST_RESULTS = None        # BassKernelResults of the last kernel() call


def _patch_drain(tc, out_dma_holder):
    """Slim TileContext's exit drain (controlled by KERNEL_DRAIN_MODE):
    'full'     stock ending (drain + EVSEM barrier + sem clear + barrier)
    'nobar2'   stock minus the trailing all-engine barrier
    'plainsem' plain-semaphore ending: the output DMA already implies all
               prior work (every instruction feeds it transitively), so the
               sync engine waits for its completion semaphore, every engine
               bumps a barrier semaphore and halts, and gpsimd (last) waits
               for the barrier then clears/resets all semaphores for NEFF
               re-execution.  Avoids the slow event-semaphore butterflies.
    """
    import os
    import re
    import types
    from concourse.vector_clock import ScopedClock

    mode = os.environ.get("KERNEL_DRAIN_MODE", "nobar2")
    if mode == "full":
        return

    def _slim(self, tick_clock, wait_clock):
        nc = self.nc
        if mode == "plainsem":
            # Replicate the final drain with a chain of single-semaphore
            # plain waits on the sync engine (sum each semaphore's program-
            # wide updates and wait for that final value) instead of the
            # multi-wait NOP that Bacc lowers to a slow event-semaphore
            # butterfly.  Then a plain-semaphore all-engine barrier, then
            # gpsimd (which passes the barrier last, after sync has observed
            # every DMA completion) clears semaphores for NEFF re-execution.
            totals = {}
            upd_re = re.compile(r"update:S\[([A-Za-z0-9_]+)\](?:\+\+|\+=)(\d+)")
            for bb in nc.main_func.blocks:
                for ins in bb.instructions:
                    for mm in upd_re.finditer(str(ins)):
                        totals[mm.group(1)] = totals.get(mm.group(1), 0) + int(mm.group(2))
            by_name = {h.name: h for h in self.sems.allocated().values()}
            waits = [(h, totals[name]) for name, h in sorted(by_name.items())
                     if totals.get(name, 0) > 0]
            for eng in nc.engines.values():
                for h, total in waits:
                    eng.wait_ge(h, total)
            popped = nc._tile_sem_poison_stack.pop()
            assert popped is self._sem_poison
            nc.clear_and_free_semaphores(
                list(self.sems.allocated().values()))
            return
        drain_inst = nc.sync.drain()
        wait_clock.add_sem_waits(
            drain_inst.ins, ScopedClock({None: tick_clock.global_clock}))
        nc.all_engine_barrier()
        popped = nc._tile_sem_poison_stack.pop()
        assert popped is self._sem_poison
        nc.clear_and_free_semaphores(list(self.sems.allocated().values()))

    tc._drain_and_barrier = types.MethodType(_slim, tc)


def _build_nc() -> bass.Bass:
    nc = bacc.Bacc()

    sz_h = nc.declare_dram_parameter("raw_sizes", [BC, S], I32, isOutput=False)
    dl_h = nc.declare_dram_parameter("raw_delays", [BC, S], F32, isOutput=False)
    dr_h = nc.declare_dram_parameter("raw_directions", [BC, S], I32, isOutput=False)
    dms_h = nc.declare_dram_parameter("delay_ms", [BC], F32, isOutput=False)
    pad_h = nc.declare_dram_parameter("padding_norm", [BC], F32, isOutput=False)
    cnf_h = nc.declare_dram_parameter("confidence", [BC], F32, isOutput=False)
    pid_h = nc.declare_dram_parameter("profile_ids", [BC], I32, isOutput=False)
    out_h = nc.declare_dram_parameter("partial", [P, 1], F32, isOutput=True)

    out_dma_holder = []
    with tile.TileContext(nc) as tc, ExitStack() as ctx:
        _patch_drain(tc, out_dma_holder)
        inp = ctx.enter_context(tc.tile_pool(name="inp", bufs=4))
        scr = ctx.enter_context(tc.tile_pool(name="scr", bufs=2))
        sm = ctx.enter_context(tc.tile_pool(name="sm", bufs=1))

        def smt(tag, dtype=F32):
            return sm.tile([P, NT], dtype, tag=tag, name=tag)

        _consts = {}

        def constv(val):
            """[128,1] f32 SBUF tile holding `val` (for activation bias APs)."""
            if val not in _consts:
                cname = f"cst{len(_consts)}"
                ct = sm.tile([P, 1], F32, tag=cname, name=cname)
                nc.vector.memset(ct[:, :], val)
                _consts[val] = ct[:, :]
            return _consts[val]

        # Row mapping: core row r -> (partition p, tile t) with r = p*NT + t.
        # This makes the per-row [128, NT] vector loads a dense 2D DMA
        # (partition stride 16B) instead of a 512-descriptor gather, while
        # the big tile loads just become row-strided (stride NT rows), which
        # costs the same descriptors as contiguous rows.
        dvec = smt("dvec")
        pvec = smt("pvec")
        cvec = smt("cvec")
        pidt = smt("pidt", I32)

        # Big-op accumulators (per tile column), split into column halves so
        # the last tile's compute tail after its DMA is only a half-op deep
        R1a, R1b = smt("R1a"), smt("R1b")   # sum sign(sz - 1400.5)
        R2a, R2b = smt("R2a"), smt("R2b")   # sum sign(0.05 - dly)
        R3a, R3b = smt("R3a"), smt("R3b")   # count sz[s] == sz[s-1], s=1..S-1
        R4a, R4b = smt("R4a"), smt("R4b")   # count dir[s] != dir[s-1], s=1..S-1
        szlast = smt("szlast")   # f32 copy of int sz[:, S-1]
        szprev = smt("szprev")   # f32 copy of int sz[:, S-2]
        dllast = smt("dllast")   # f32 copy of dly[:, S-1]

        sz_t = sz_h[:, :].rearrange("(p t) s -> t p s", t=NT)
        dl_t = dl_h[:, :].rearrange("(p t) s -> t p s", t=NT)
        dr_t = dr_h[:, :].rearrange("(p t) s -> t p s", t=NT)
        H = S // 2
        for t in range(NT):
            szt = inp.tile([P, S], I32, tag="szt")
            dlt = inp.tile([P, S], F32, tag="dlt")
            drt = inp.tile([P, S], I32, tag="drt")
            for h, cs in ((0, slice(0, H)), (1, slice(H, S))):
                nc.sync.dma_start(szt[:, cs], sz_t[t][:, cs])
                nc.sync.dma_start(dlt[:, cs], dl_t[t][:, cs])
                nc.sync.dma_start(drt[:, cs], dr_t[t][:, cs])

            col = slice(t, t + 1)
            o1 = scr.tile([P, S], F32, tag="o1")
            nc.scalar.activation(o1[:, 0:H], szt[:, 0:H], ACTF.Sign,
                                 bias=constv(-1400.5), scale=1.0, accum_out=R1a[:, col])
            nc.scalar.activation(o1[:, H:S], szt[:, H:S], ACTF.Sign,
                                 bias=constv(-1400.5), scale=1.0, accum_out=R1b[:, col])
            o2 = scr.tile([P, S], F32, tag="o2")
            nc.scalar.activation(o2[:, 0:H], dlt[:, 0:H], ACTF.Sign,
                                 bias=constv(0.05), scale=-1.0, accum_out=R2a[:, col])
            nc.scalar.activation(o2[:, H:S], dlt[:, H:S], ACTF.Sign,
                                 bias=constv(0.05), scale=-1.0, accum_out=R2b[:, col])
            # fused compare + row-sum on DVE: out = (in0 bypass 0) cmp in1,
            # accum_out = sum(out).  (tensor_tensor_reduce crashes the HW
            # runtime in this toolchain; scalar_tensor_tensor w/ accum works.)
            o3 = scr.tile([P, S - 1], F32, tag="o3")
            nc.vector.scalar_tensor_tensor(
                o3[:, 0:H - 1], szt[:, 1:H], 0.0, szt[:, 0:H - 1],
                ALU.bypass, ALU.is_equal, accum_out=R3a[:, col])
            nc.vector.scalar_tensor_tensor(
                o3[:, H - 1:S - 1], szt[:, H:S], 0.0, szt[:, H - 1:S - 1],
                ALU.bypass, ALU.is_equal, accum_out=R3b[:, col])
            o4 = scr.tile([P, S - 1], F32, tag="o4")
            nc.vector.scalar_tensor_tensor(
                o4[:, 0:H - 1], drt[:, 1:H], 0.0, drt[:, 0:H - 1],
                ALU.bypass, ALU.not_equal, accum_out=R4a[:, col])
            nc.vector.scalar_tensor_tensor(
                o4[:, H - 1:S - 1], drt[:, H:S], 0.0, drt[:, H - 1:S - 1],
                ALU.bypass, ALU.not_equal, accum_out=R4b[:, col])

            nc.vector.tensor_copy(szlast[:, col], szt[:, S - 1:S])
            nc.vector.tensor_copy(szprev[:, col], szt[:, S - 2:S - 1])
            nc.vector.tensor_copy(dllast[:, col], dlt[:, S - 1:S])

        # per-row vectors loaded after the big streams are queued (tiny DMAs)
        nc.gpsimd.dma_start(dvec[:, :], dms_h[:].rearrange("(p t) -> p t", t=NT))
        nc.gpsimd.dma_start(pvec[:, :], pad_h[:].rearrange("(p t) -> p t", t=NT))
        nc.gpsimd.dma_start(cvec[:, :], cnf_h[:].rearrange("(p t) -> p t", t=NT))
        nc.gpsimd.dma_start(pidt[:, :], pid_h[:].rearrange("(p t) -> p t", t=NT))

        # ---- per-sample combine, all on [128, 4] tiles (VectorE only,
        # to keep per-instruction sync-wait counts low on ScalarE) ----
        v = nc.vector

        # merge column-half accumulators
        R1, R2, R3, R4 = smt("R1"), smt("R2"), smt("R3"), smt("R4")
        v.tensor_add(R1[:, :], R1a[:, :], R1b[:, :])
        v.tensor_add(R2[:, :], R2a[:, :], R2b[:, :])
        v.tensor_add(R3[:, :], R3a[:, :], R3b[:, :])
        v.tensor_add(R4[:, :], R4a[:, :], R4b[:, :])

        # profile-id one-hots (pid in 0..4)
        pidf = smt("pidf")
        v.tensor_copy(pidf[:, :], pidt[:, :])
        e1, e2, e3, e4 = smt("e1"), smt("e2"), smt("e3"), smt("e4")
        v.tensor_scalar(e1[:, :], pidf[:, :], 1.0, None, ALU.is_equal)
        v.tensor_scalar(e2[:, :], pidf[:, :], 2.0, None, ALU.is_equal)
        v.tensor_scalar(e3[:, :], pidf[:, :], 3.0, None, ALU.is_equal)
        v.tensor_scalar(e4[:, :], pidf[:, :], 4.0, None, ALU.is_equal)

        # CONFIG_MULT[pid % 4] = 1.0 + 0.3*e1 + 0.6*e2 + 1.0*e3  (pid=4 -> 1.0)
        mlt = smt("mlt")
        v.tensor_scalar(mlt[:, :], e1[:, :], 0.3, 1.0, ALU.mult, ALU.add)
        v.scalar_tensor_tensor(mlt[:, :], e2[:, :], 0.6, mlt[:, :], ALU.mult, ALU.add)
        v.tensor_add(mlt[:, :], mlt[:, :], e3[:, :])

        # TARGET_DELAY[pid] = 2 - 1*e1 - 1.5*e2 + 3*e3 + 1*e4
        td = smt("td")
        v.tensor_scalar(td[:, :], e1[:, :], -1.0, 2.0, ALU.mult, ALU.add)
        v.scalar_tensor_tensor(td[:, :], e2[:, :], -1.5, td[:, :], ALU.mult, ALU.add)
        v.scalar_tensor_tensor(td[:, :], e3[:, :], 3.0, td[:, :], ALU.mult, ALU.add)
        v.tensor_add(td[:, :], td[:, :], e4[:, :])

        # TARGET_PAD[pid] = 0.08 + 0.04*e1 - 0.03*e2 + 0.07*e3 + 0.02*e4
        tp = smt("tp")
        v.tensor_scalar(tp[:, :], e1[:, :], 0.04, 0.08, ALU.mult, ALU.add)
        v.scalar_tensor_tensor(tp[:, :], e2[:, :], -0.03, tp[:, :], ALU.mult, ALU.add)
        v.scalar_tensor_tensor(tp[:, :], e3[:, :], 0.07, tp[:, :], ALU.mult, ALU.add)
        v.scalar_tensor_tensor(tp[:, :], e4[:, :], 0.02, tp[:, :], ALU.mult, ALU.add)

        # last-column morphing fixups
        padx = smt("padx")
        v.tensor_scalar(padx[:, :], pvec[:, :], 1500.0, None, ALU.mult)
        szmod = smt("szmod")
        v.tensor_add(szmod[:, :], szlast[:, :], padx[:, :])
        v.tensor_scalar(szmod[:, :], szmod[:, :], 1500.0, None, ALU.min)
        dlmod = smt("dlmod")
        v.tensor_add(dlmod[:, :], dllast[:, :], dvec[:, :])

        g1m, g1r = smt("g1m"), smt("g1r")
        v.tensor_scalar(g1m[:, :], szmod[:, :], 1400.0, None, ALU.is_gt)
        v.tensor_scalar(g1r[:, :], szlast[:, :], 1400.0, None, ALU.is_gt)
        l2m, l2r = smt("l2m"), smt("l2r")
        v.tensor_scalar(l2m[:, :], dlmod[:, :], 0.05, None, ALU.is_lt)
        v.tensor_scalar(l2r[:, :], dllast[:, :], 0.05, None, ALU.is_lt)
        e3r = smt("e3r")
        v.tensor_tensor(e3r[:, :], szlast[:, :], szprev[:, :], ALU.is_equal)
        d3 = smt("d3")
        v.tensor_sub(d3[:, :], szmod[:, :], szprev[:, :])
        a3 = smt("a3")
        nc.scalar.activation(a3[:, :], d3[:, :], ACTF.Abs)
        e3m = smt("e3m")
        v.tensor_scalar(e3m[:, :], a3[:, :], 0.5, None, ALU.is_lt)

        # exact per-row counts
        cnt1 = smt("cnt1")
        v.tensor_scalar(cnt1[:, :], R1[:, :], 0.5, float(S) / 2, ALU.mult, ALU.add)
        v.tensor_sub(cnt1[:, :], cnt1[:, :], g1r[:, :])
        v.tensor_add(cnt1[:, :], cnt1[:, :], g1m[:, :])
        cnt2 = smt("cnt2")
        v.tensor_scalar(cnt2[:, :], R2[:, :], 0.5, float(S) / 2, ALU.mult, ALU.add)
        v.tensor_sub(cnt2[:, :], cnt2[:, :], l2r[:, :])
        v.tensor_add(cnt2[:, :], cnt2[:, :], l2m[:, :])
        cnt3 = smt("cnt3")
        v.tensor_sub(cnt3[:, :], R3[:, :], e3r[:, :])
        v.tensor_add(cnt3[:, :], cnt3[:, :], e3m[:, :])

        # scores = (0.6*c1 + 0.4*c2 + 0.2*c3 + 0.1*c4 + 0.1) * (100/S) * mult
        acc = smt("acc")
        v.tensor_scalar(acc[:, :], cnt1[:, :], 0.6, None, ALU.mult)
        v.scalar_tensor_tensor(acc[:, :], cnt2[:, :], 0.4, acc[:, :], ALU.mult, ALU.add)
        v.scalar_tensor_tensor(acc[:, :], cnt3[:, :], 0.2, acc[:, :], ALU.mult, ALU.add)
        v.scalar_tensor_tensor(acc[:, :], R4[:, :], 0.1, acc[:, :], ALU.mult, ALU.add)
        base = smt("base")
        v.tensor_scalar(base[:, :], acc[:, :], 100.0 / S, 0.1 * 100.0 / S,
                        ALU.mult, ALU.add)
        scores = smt("scores")
        v.tensor_mul(scores[:, :], base[:, :], mlt[:, :])

        ev = smt("ev")
        v.tensor_scalar(ev[:, :], scores[:, :], 30.0, None, ALU.is_lt)
        dpi = smt("dpi")
        v.tensor_scalar(dpi[:, :], scores[:, :], 15.0, -15.0, ALU.max, ALU.add)

        sd = smt("sd")
        v.tensor_sub(sd[:, :], dvec[:, :], td[:, :])
        sda = smt("sda")
        nc.scalar.activation(sda[:, :], sd[:, :], ACTF.Abs)
        sp = smt("sp")
        v.tensor_sub(sp[:, :], pvec[:, :], tp[:, :])
        spa = smt("spa")
        nc.scalar.activation(spa[:, :], sp[:, :], ACTF.Abs)
        sim = smt("sim")
        v.tensor_add(sim[:, :], sda[:, :], spa[:, :])

        ed = smt("ed")
        v.tensor_scalar(ed[:, :], dvec[:, :], 20.0, -20.0, ALU.max, ALU.add)
        ep = smt("ep")
        v.tensor_scalar(ep[:, :], pvec[:, :], 0.3, -0.3, ALU.max, ALU.add)
        eff = smt("eff")
        v.scalar_tensor_tensor(eff[:, :], ed[:, :], 1.0 / 20.0, ep[:, :],
                               ALU.mult, ALU.add)

        cd = smt("cd")
        v.tensor_sub(cd[:, :], cvec[:, :], ev[:, :])
        cq = smt("cq")
        v.tensor_mul(cq[:, :], cd[:, :], cd[:, :])

        ctot = smt("ctot")
        v.tensor_scalar(ctot[:, :], dpi[:, :], 2.0 / 30.0, None, ALU.mult)
        v.scalar_tensor_tensor(ctot[:, :], sim[:, :], 0.5, ctot[:, :], ALU.mult, ALU.add)
        v.scalar_tensor_tensor(ctot[:, :], eff[:, :], 0.3, ctot[:, :], ALU.mult, ALU.add)
        v.scalar_tensor_tensor(ctot[:, :], cq[:, :], 0.2, ctot[:, :], ALU.mult, ALU.add)

        red = sm.tile([P, 1], F32, tag="red", name="red")
        v.tensor_reduce(red[:, :], ctot[:, :], axis=mybir.AxisListType.X, op=ALU.add)
        out_dma_holder.append(nc.sync.dma_start(out_h[:, :], red[:, :]))

    nc.finalize()
    return nc


def _get_nc() -> bass.Bass:
    global _NC_CACHE
    if _NC_CACHE is None:
        _NC_CACHE = _build_nc()
    return _NC_CACHE


def kernel(raw_sizes, raw_delays, raw_directions, delay_ms, padding_norm,
           confidence, profile_ids, trace=False, tmpdir=None):
    global LAST_RESULTS
    raw_sizes = np.asarray(raw_sizes, dtype=np.int32)
    raw_delays = np.asarray(raw_delays, dtype=np.float32)
    raw_directions = np.asarray(raw_directions, dtype=np.int32)
    delay_ms = np.asarray(delay_ms, dtype=np.float32)
    padding_norm = np.asarray(padding_norm, dtype=np.float32)
    confidence = np.asarray(confidence, dtype=np.float32)
    profile_ids = np.asarray(profile_ids).astype(np.int32)

    nc = _get_nc()
    in_maps = []
    for i in range(N_CORES):
        r = slice(i * BC, (i + 1) * BC)
        in_maps.append({
            "raw_sizes": raw_sizes[r],
            "raw_delays": raw_delays[r],
            "raw_directions": raw_directions[r],
            "delay_ms": delay_ms[r],
            "padding_norm": padding_norm[r],
            "confidence": confidence[r],
            "profile_ids": profile_ids[r],
        })

    LAST_RESULTS = run_bass_kernel_spmd(nc, in_maps, list(range(N_CORES)),
                                        trace=trace, tmpdir=tmpdir)
    partials = [LAST_RESULTS.results[i]["partial"] for i in range(N_CORES)]
    total = float(np.sum(np.stack(partials), dtype=np.float64))
    return np.float32(total / B)



# revision 11
# speedup vs baseline: 1.1624x; 1.1624x over previous
"""AdversarialMorphingLoss — Trainium2 Bass kernel (8-core data parallel).

Full inputs arrive on the host; we shard the batch dim (B=4096) into 8
contiguous blocks of 512 rows, run one SPMD Bass program on all 8
NeuronCores, and each core returns its 512 per-row loss contributions
as a [128, 4] f32 tile.  The host sums everything and divides by B.

Per-sample math (matching reference.py):
  scores_b = 100/S * sum_s inc_s * CONFIG_MULT[pid % 4]
  inc_s    = 0.6*(sz_s > 1400) + 0.4*(dly_s < 0.05)
           + 0.2*(|sz_s - sz_{s-1}| < 0.5) + 0.1*(dir_s != dir_{s-1})
  with sz[:, -1] -> min(sz[:, -1] + pad*1500, 1500), dly[:, -1] += delay_ms,
  and the s=0 "prev" being -1.0 (so the dir term contributes 0.1 at s=0 and
  the size-equality term contributes 0).

  c_b = (2/30)*relu(scores-15) + 0.5*(|dly_ms - TD[pid]| + |pad - TP[pid]|)
      + 0.3*(relu(dly_ms-20)/20 + relu(pad-0.3)) + 0.2*(conf - (scores<30))^2
  loss = mean_b c_b

Performance structure (memory-bound: streams 12 MB/core of traces):
  * Row mapping r = p*NT + t so per-row [128, NT] vectors are dense DMAs.
  * Tiles 0-2: one full [128, 2048] DMA per tensor; ScalarE counts
    (sz > 1400) and (dly < 0.05) via Sign activations with accum_out,
    VectorE counts adjacent-size equality via fused STT+accum, gpsimd
    (Pool) counts direction flips the same way.  All three engines run
    well under the ~7.3us/tile DMA time, so the stream stays saturated.
  * Tile 3 is DMA'd in 4 chunks with the LAST columns first, so the
    last-packet morphing fixups (needing cols S-2, S-1) and every other
    per-row quantity are computed mid-stream; the final chunk is only
    256 columns, leaving a sub-us compute tail after the last DMA.
  * The per-row combine is algebraically folded into one short weighted
    chain; (conf - ev)^2 is expanded so `ev` feeds a single mult.
  * The TileContext exit drain is replaced by a chain of single-wait
    drains (native, fast busy-poll) instead of the stock multi-wait
    event-semaphore barrier which costs ~10us of wake-up latency.
"""

import numpy as np
from contextlib import ExitStack

import concourse.bass as bass
import concourse.bacc as bacc
import concourse.mybir as mybir
from concourse import tile
from concourse.bass_utils import run_bass_kernel_spmd

B, S = 4096, 2048
N_CORES = 8
BC = B // N_CORES          # 512 rows per core
P = 128                    # SBUF partitions
NT = BC // P               # 4 tiles of 128 rows per core

F32 = mybir.dt.float32
I32 = mybir.dt.int32
ALU = mybir.AluOpType
ACTF = mybir.ActivationFunctionType

# tile-3 column chunks, in DMA order: the chunk holding the last two
# columns goes FIRST so the morphing fixups can run mid-stream; the
# 256-column chunk processed last minimizes the post-stream tail.
T3_CHUNKS = [(1792, 2048), (0, 768), (768, 1536), (1536, 1792)]
LAST_CS, LAST_CE = 1536, 1792

_NC_CACHE = None
LAST_RESULTS = None        # BassKernelResults of the last kernel() call


def _patch_drain(tc, out_dma_holder):
    """Replace TileContext's exit drain (KERNEL_DRAIN_MODE):
    'full'   stock ending (multi-wait drain + EVSEM barrier + sem clear
             + barrier) — costs ~14us of event-semaphore wake latency.
    'nobar2' stock minus the trailing all-engine barrier.
    'slim'   (default) chain of single-wait DRAINs on the sync engine —
             one wait per instruction stays native (fast busy-poll)
             instead of being split into event-semaphore NOPs — then
             sync bumps a signal sem and gpsimd (after one native wait)
             range-clears all tile semaphores for NEFF re-execution.
    """
    import os
    import re
    import types

    mode = os.environ.get("KERNEL_DRAIN_MODE", "slim")
    if mode == "full":
        return

    def _slim(self, tick_clock, wait_clock):
        from concourse.vector_clock import ScopedClock

        nc = self.nc
        if mode == "slim":
            # Final value of every tile semaphore = sum of its updates.
            totals = {}
            upd_re = re.compile(r"update:S\[([A-Za-z0-9_]+)\](?:\+\+|\+=)(\d+)")
            for bb in nc.main_func.blocks:
                for ins in bb.instructions:
                    for mm in upd_re.finditer(str(ins)):
                        totals[mm.group(1)] = totals.get(mm.group(1), 0) + int(mm.group(2))
            by_name = {h.name: h for h in self.sems.allocated().values()}
            waits = [(h, totals[name]) for name, h in sorted(by_name.items())
                     if totals.get(name, 0) > 0]
            # sync observes every semaphore reaching its final value (this
            # covers the output DMA completion), one wait per DRAIN so the
            # waits stay attached (hardware allows 1 native wait/inst).
            for h, total in waits:
                nc.sync.drain().wait_op(h, total, "sem-ge")
            sig, sig_total = waits[0]
            nc.sync.drain().then_inc(sig, 1)
            # gpsimd: one native wait on the signal, then bulk-clear.
            nc.gpsimd.drain().wait_op(sig, sig_total + 1, "sem-ge")
            popped = nc._tile_sem_poison_stack.pop()
            assert popped is self._sem_poison
            nc.clear_and_free_semaphores(list(self.sems.allocated().values()))
            return
        # 'nobar2': stock ending minus the trailing all-engine barrier
        drain_inst = nc.sync.drain()
        wait_clock.add_sem_waits(
            drain_inst.ins, ScopedClock({None: tick_clock.global_clock}))
        nc.all_engine_barrier()
        popped = nc._tile_sem_poison_stack.pop()
        assert popped is self._sem_poison
        nc.clear_and_free_semaphores(list(self.sems.allocated().values()))

    tc._drain_and_barrier = types.MethodType(_slim, tc)


def _build_nc() -> bass.Bass:
    nc = bacc.Bacc()

    sz_h = nc.declare_dram_parameter("raw_sizes", [BC, S], I32, isOutput=False)
    dl_h = nc.declare_dram_parameter("raw_delays", [BC, S], F32, isOutput=False)
    dr_h = nc.declare_dram_parameter("raw_directions", [BC, S], I32, isOutput=False)
    dms_h = nc.declare_dram_parameter("delay_ms", [BC], F32, isOutput=False)
    pad_h = nc.declare_dram_parameter("padding_norm", [BC], F32, isOutput=False)
    cnf_h = nc.declare_dram_parameter("confidence", [BC], F32, isOutput=False)
    pid_h = nc.declare_dram_parameter("profile_ids", [BC], I32, isOutput=False)
    out_h = nc.declare_dram_parameter("partial", [P, NT], F32, isOutput=True)

    out_dma_holder = []
    with tile.TileContext(nc) as tc, ExitStack() as ctx:
        _patch_drain(tc, out_dma_holder)
        inp = ctx.enter_context(tc.tile_pool(name="inp", bufs=4))
        scr = ctx.enter_context(tc.tile_pool(name="scr", bufs=2))
        sm = ctx.enter_context(tc.tile_pool(name="sm", bufs=1))

        V = nc.vector
        Pl = nc.gpsimd
        Sc = nc.scalar

        def smt(tag, dtype=F32):
            return sm.tile([P, NT], dtype, tag=tag, name=tag)

        _consts = {}

        def constv(val):
            """[128,1] f32 SBUF tile holding `val` (for activation bias APs)."""
            if val not in _consts:
                cname = f"cst{len(_consts)}"
                ct = sm.tile([P, 1], F32, tag=cname, name=cname)
                V.memset(ct[:, :], val)
                _consts[val] = ct[:, :]
            return _consts[val]

        # ---- per-row vectors: dense [128, NT] DMAs (row r = p*NT + t) ----
        dvec, pvec, cvec = smt("dvec"), smt("pvec"), smt("cvec")
        pidt = smt("pidt", I32)
        Pl.dma_start(dvec[:, :], dms_h[:].rearrange("(p t) -> p t", t=NT))
        Pl.dma_start(pvec[:, :], pad_h[:].rearrange("(p t) -> p t", t=NT))
        Pl.dma_start(cvec[:, :], cnf_h[:].rearrange("(p t) -> p t", t=NT))
        Pl.dma_start(pidt[:, :], pid_h[:].rearrange("(p t) -> p t", t=NT))

        # ---- big input DMAs (sync engine).  Tile-3's last-column chunk
        # is issued FIRST, then tiles 0-2 whole, then tile-3's remainder.
        sz_t = sz_h[:, :].rearrange("(p t) s -> t p s", t=NT)
        dl_t = dl_h[:, :].rearrange("(p t) s -> t p s", t=NT)
        dr_t = dr_h[:, :].rearrange("(p t) s -> t p s", t=NT)

        tiles = {}
        for t in range(NT):
            tiles[t] = (inp.tile([P, S], I32, tag="szt", name=f"szt{t}"),
                        inp.tile([P, S], F32, tag="dlt", name=f"dlt{t}"),
                        inp.tile([P, S], I32, tag="drt", name=f"drt{t}"))

        def dma_tile(t, cs, ce):
            szt, dlt, drt = tiles[t]
            c = slice(cs, ce)
            nc.sync.dma_start(szt[:, c], sz_t[t][:, c])
            nc.sync.dma_start(dlt[:, c], dl_t[t][:, c])
            nc.sync.dma_start(drt[:, c], dr_t[t][:, c])

        dma_tile(3, *T3_CHUNKS[0])           # cols 1792:2048 first
        for t in range(3):
            dma_tile(t, 0, S)
        for cs, ce in T3_CHUNKS[1:]:
            dma_tile(3, cs, ce)

        # ---- accumulators.  A: full tiles 0-2 + tile-3 chunk a.
        # B/D: tile-3 chunks b/d (cols 0-2 memset 0).  C1s: tile-3 sign
        # chunk c for sizes (Scalar).  C2d: direct (dly<0.05) count for
        # chunk c (Pool, vs a 0.05 const tile).  C3/C4: chunk-c pair
        # counts (Vector/Pool).
        def acc(tag, zero):
            tl = smt(tag)
            if zero:
                Pl.memset(tl[:, :], 0.0)
            return tl

        A1, B1, C1s, D1 = acc("A1", 0), acc("B1", 1), acc("C1s", 1), acc("D1", 1)
        A2, B2, C2s, D2 = acc("A2", 0), acc("B2", 1), acc("C2s", 1), acc("D2", 1)
        A3, B3, C3, D3 = acc("A3", 0), acc("B3", 1), acc("C3", 1), acc("D3", 1)
        A4, B4, C4, D4 = acc("A4", 0), acc("B4", 1), acc("C4", 1), acc("D4", 1)

        # last/prev column copies (f32), per tile; tile 3's come from the
        # early chunk so everything below runs mid-stream.
        szlast, szprev, dllast = smt("szlast"), smt("szprev"), smt("dllast")

        oS = scr.tile([P, S], F32, tag="oS", name="oS")    # dead outputs, per engine
        oV = scr.tile([P, S], F32, tag="oV", name="oV")
        oP = scr.tile([P, S], F32, tag="oP", name="oP")

        def compute_chunk(t, cs, ce, p0, p1, accs):
            """Sign-count cols [cs,ce); pair-count pairs s in [p0,p1).
            accs = (q1, q2, q3, q4); a pair (s-1, s) reads cols s-1 and s,
            so [p0,p1) must only touch columns already DMA'd for tile t."""
            szt, dlt, drt = tiles[t]
            col = slice(t, t + 1)
            q1, q2, q3, q4 = accs
            Sc.activation(oS[:, cs:ce], szt[:, cs:ce], ACTF.Sign,
                          bias=constv(-1400.5), scale=1.0, accum_out=q1[:, col])
            Sc.activation(oS[:, cs:ce], dlt[:, cs:ce], ACTF.Sign,
                          bias=constv(0.05), scale=-1.0, accum_out=q2[:, col])
            V.scalar_tensor_tensor(
                oV[:, p0:p1], szt[:, p0:p1], 0.0, szt[:, p0 - 1:p1 - 1],
                ALU.bypass, ALU.is_equal, accum_out=q3[:, col])
            V.scalar_tensor_tensor(
                oV[:, p0:p1], drt[:, p0:p1], 0.0, drt[:, p0 - 1:p1 - 1],
                ALU.bypass, ALU.not_equal, accum_out=q4[:, col])

        def copy_lastcols(t):
            szt, dlt, _ = tiles[t]
            col = slice(t, t + 1)
            Pl.tensor_copy(szlast[:, col], szt[:, S - 1:S])
            Pl.tensor_copy(szprev[:, col], szt[:, S - 2:S - 1])
            Pl.tensor_copy(dllast[:, col], dlt[:, S - 1:S])

        # tile 3, chunk d (first data to land); pairs start at s=1793 so
        # col 1792 is the only boundary column it touches (self-contained)
        copy_lastcols(3)
        compute_chunk(3, 1792, 2048, 1793, 2048, (D1, D2, D3, D4))

        # tiles 0-2, full
        for t in range(3):
            copy_lastcols(t)
            compute_chunk(t, 0, S, 1, S, (A1, A2, A3, A4))

        # ---- early per-row math (depends only on small vectors) ----
        pidf = smt("pidf")
        V.tensor_copy(pidf[:, :], pidt[:, :])
        e1, e2, e3, e4 = smt("e1"), smt("e2"), smt("e3"), smt("e4")
        V.tensor_scalar(e1[:, :], pidf[:, :], 1.0, None, ALU.is_equal)
        V.tensor_scalar(e2[:, :], pidf[:, :], 2.0, None, ALU.is_equal)
        V.tensor_scalar(e3[:, :], pidf[:, :], 3.0, None, ALU.is_equal)
        V.tensor_scalar(e4[:, :], pidf[:, :], 4.0, None, ALU.is_equal)

        # mlt2 = CONFIG_MULT[pid % 4] * (100/S)
        ms = 100.0 / S
        mlt2 = smt("mlt2")
        V.tensor_scalar(mlt2[:, :], e1[:, :], 0.3 * ms, ms, ALU.mult, ALU.add)
        V.scalar_tensor_tensor(mlt2[:, :], e2[:, :], 0.6 * ms, mlt2[:, :],
                               ALU.mult, ALU.add)
        V.scalar_tensor_tensor(mlt2[:, :], e3[:, :], 1.0 * ms, mlt2[:, :],
                               ALU.mult, ALU.add)

        # TARGET_DELAY[pid] = 2 - e1 - 1.5 e2 + 3 e3 + e4
        td = smt("td")
        V.tensor_scalar(td[:, :], e1[:, :], -1.0, 2.0, ALU.mult, ALU.add)
        V.scalar_tensor_tensor(td[:, :], e2[:, :], -1.5, td[:, :], ALU.mult, ALU.add)
        V.scalar_tensor_tensor(td[:, :], e3[:, :], 3.0, td[:, :], ALU.mult, ALU.add)
        V.tensor_add(td[:, :], td[:, :], e4[:, :])

        # TARGET_PAD[pid] = 0.08 + 0.04 e1 - 0.03 e2 + 0.07 e3 + 0.02 e4
        tp = smt("tp")
        V.tensor_scalar(tp[:, :], e1[:, :], 0.04, 0.08, ALU.mult, ALU.add)
        V.scalar_tensor_tensor(tp[:, :], e2[:, :], -0.03, tp[:, :], ALU.mult, ALU.add)
        V.scalar_tensor_tensor(tp[:, :], e3[:, :], 0.07, tp[:, :], ALU.mult, ALU.add)
        V.scalar_tensor_tensor(tp[:, :], e4[:, :], 0.02, tp[:, :], ALU.mult, ALU.add)

        # Bfin = 0.5(|dms-td| + |pad-tp|) + 0.015 relu(dms-20)
        #      + 0.3 relu(pad-0.3) + 0.2 conf^2 - 1
        sd = smt("sd")
        V.tensor_sub(sd[:, :], dvec[:, :], td[:, :])
        sda = smt("sda")
        Sc.activation(sda[:, :], sd[:, :], ACTF.Abs)
        sp = smt("sp")
        V.tensor_sub(sp[:, :], pvec[:, :], tp[:, :])
        spa = smt("spa")
        Sc.activation(spa[:, :], sp[:, :], ACTF.Abs)
        b1 = smt("b1")
        V.tensor_add(b1[:, :], sda[:, :], spa[:, :])
        Bfin = smt("Bfin")
        V.tensor_scalar(Bfin[:, :], b1[:, :], 0.5, -1.0, ALU.mult, ALU.add)
        b2 = smt("b2")
        V.tensor_scalar(b2[:, :], dvec[:, :], 20.0, -20.0, ALU.max, ALU.add)
        V.scalar_tensor_tensor(Bfin[:, :], b2[:, :], 0.3 / 20.0, Bfin[:, :],
                               ALU.mult, ALU.add)
        b3 = smt("b3")
        V.tensor_scalar(b3[:, :], pvec[:, :], 0.3, -0.3, ALU.max, ALU.add)
        V.scalar_tensor_tensor(Bfin[:, :], b3[:, :], 0.3, Bfin[:, :],
                               ALU.mult, ALU.add)
        cc = smt("cc")
        V.tensor_mul(cc[:, :], cvec[:, :], cvec[:, :])
        V.scalar_tensor_tensor(Bfin[:, :], cc[:, :], 0.2, Bfin[:, :],
                               ALU.mult, ALU.add)
        E2 = smt("E2")
        V.tensor_scalar(E2[:, :], cvec[:, :], -0.4, 0.2, ALU.mult, ALU.add)

        # ---- last-packet morphing fixups (mid-stream; [128, NT]) ----
        padx = smt("padx")
        V.tensor_scalar(padx[:, :], pvec[:, :], 1500.0, None, ALU.mult)
        szmodA = smt("szmodA")           # szlast + pad*1500 (pre-min)
        V.tensor_add(szmodA[:, :], szlast[:, :], padx[:, :])
        g1m = smt("g1m")                 # min(.,1500)>1400 == (.)>1400
        V.tensor_scalar(g1m[:, :], szmodA[:, :], 1400.0, None, ALU.is_gt)
        g1r = smt("g1r")
        V.tensor_scalar(g1r[:, :], szlast[:, :], 1400.0, None, ALU.is_gt)
        dlmod = smt("dlmod")
        V.tensor_add(dlmod[:, :], dllast[:, :], dvec[:, :])
        l2m = smt("l2m")
        V.tensor_scalar(l2m[:, :], dlmod[:, :], 0.05, None, ALU.is_lt)
        l2r = smt("l2r")
        V.tensor_scalar(l2r[:, :], dllast[:, :], 0.05, None, ALU.is_lt)
        szmodM = smt("szmodM")
        V.tensor_scalar(szmodM[:, :], szmodA[:, :], 1500.0, None, ALU.min)
        d3 = smt("d3")
        V.tensor_sub(d3[:, :], szmodM[:, :], szprev[:, :])
        a3 = smt("a3")
        Sc.activation(a3[:, :], d3[:, :], ACTF.Abs)
        e3m = smt("e3m")
        V.tensor_scalar(e3m[:, :], a3[:, :], 0.5, None, ALU.is_lt)
        e3r = smt("e3r")
        V.tensor_tensor(e3r[:, :], szlast[:, :], szprev[:, :], ALU.is_equal)

        pt1 = smt("pt1")                 # g1m - g1r
        V.scalar_tensor_tensor(pt1[:, :], g1r[:, :], -1.0, g1m[:, :],
                               ALU.mult, ALU.add)
        pt2 = smt("pt2")
        V.scalar_tensor_tensor(pt2[:, :], l2r[:, :], -1.0, l2m[:, :],
                               ALU.mult, ALU.add)
        pt3 = smt("pt3")
        V.scalar_tensor_tensor(pt3[:, :], e3r[:, :], -1.0, e3m[:, :],
                               ALU.mult, ALU.add)
        DK = smt("DK")        # (S/2)(0.6+0.4) + 0.1 + 0.6 pt1 + 0.4 pt2 + 0.2 pt3
        V.tensor_scalar(DK[:, :], pt1[:, :], 0.6, S / 2.0 + 0.1,
                        ALU.mult, ALU.add)
        V.scalar_tensor_tensor(DK[:, :], pt2[:, :], 0.4, DK[:, :],
                               ALU.mult, ALU.add)
        V.scalar_tensor_tensor(DK[:, :], pt3[:, :], 0.2, DK[:, :],
                               ALU.mult, ALU.add)

        # tile 3 chunks a, b (mid/late stream)
        compute_chunk(3, 0, 768, 1, 768, (A1, A2, A3, A4))
        compute_chunk(3, 768, 1536, 768, 1536, (B1, B2, B3, B4))

        # mid-stream partial merges (available once chunk b is done)
        S1p, S2p, Ep, Fp = smt("S1p"), smt("S2p"), smt("Ep"), smt("Fp")
        V.tensor_add(S1p[:, :], A1[:, :], B1[:, :])
        V.tensor_add(S1p[:, :], S1p[:, :], D1[:, :])
        V.tensor_add(S2p[:, :], A2[:, :], B2[:, :])
        V.tensor_add(S2p[:, :], S2p[:, :], D2[:, :])
        V.tensor_add(Ep[:, :], A3[:, :], B3[:, :])
        V.tensor_add(Ep[:, :], Ep[:, :], D3[:, :])
        V.tensor_add(Fp[:, :], A4[:, :], B4[:, :])
        V.tensor_add(Fp[:, :], Fp[:, :], D4[:, :])

        # tile 3 chunk c: 256-col tail.  sz sign on Scalar, dly direct
        # (< 0.05 vs const tile) on Pool, pairs on Vector/Pool.
        szt3, dlt3, drt3 = tiles[3]
        cs, ce = LAST_CS, LAST_CE
        col3 = slice(3, 4)
        Sc.activation(oS[:, cs:ce], szt3[:, cs:ce], ACTF.Sign,
                      bias=constv(-1400.5), scale=1.0, accum_out=C1s[:, col3])
        Sc.activation(oS[:, cs:ce], dlt3[:, cs:ce], ACTF.Sign,
                      bias=constv(0.05), scale=-1.0, accum_out=C2s[:, col3])
        # pairs s in [1536, 1793): both boundary pairs (cols 1535 from
        # chunk b, col 1792 from chunk d) are already resident.
        V.scalar_tensor_tensor(
            oV[:, cs:ce + 1], szt3[:, cs:ce + 1], 0.0, szt3[:, cs - 1:ce],
            ALU.bypass, ALU.is_equal, accum_out=C3[:, col3])
        V.scalar_tensor_tensor(
            oV[:, cs:ce + 1], drt3[:, cs:ce + 1], 0.0, drt3[:, cs - 1:ce],
            ALU.bypass, ALU.not_equal, accum_out=C4[:, col3])

        # ---- tail combine ----
        # inc_sum = 0.2*(1.5*S1 + S2 + E + 0.5*F) + DK      (S1/S2 signed)
        S1, S2, E, F = smt("S1"), smt("S2"), smt("E"), smt("F")
        V.tensor_add(S1[:, :], S1p[:, :], C1s[:, :])
        V.tensor_add(S2[:, :], S2p[:, :], C2s[:, :])
        V.tensor_add(E[:, :], Ep[:, :], C3[:, :])
        V.tensor_add(F[:, :], Fp[:, :], C4[:, :])
        t12 = smt("t12")
        V.scalar_tensor_tensor(t12[:, :], S1[:, :], 1.5, S2[:, :],
                               ALU.mult, ALU.add)
        t34 = smt("t34")
        V.scalar_tensor_tensor(t34[:, :], F[:, :], 0.5, E[:, :],
                               ALU.mult, ALU.add)
        z0 = smt("z0")
        V.tensor_add(z0[:, :], t12[:, :], t34[:, :])
        W = smt("W")
        V.scalar_tensor_tensor(W[:, :], z0[:, :], 0.2, DK[:, :],
                               ALU.mult, ALU.add)
        scores = smt("scores")
        V.tensor_mul(scores[:, :], W[:, :], mlt2[:, :])

        ev = smt("ev")
        V.tensor_scalar(ev[:, :], scores[:, :], 30.0, None, ALU.is_lt)
        u = smt("u")                      # max(scores,15)/15 (dpi term + 1)
        V.tensor_scalar(u[:, :], scores[:, :], 15.0, 1.0 / 15.0,
                        ALU.max, ALU.mult)
        m = smt("m")
        V.tensor_mul(m[:, :], ev[:, :], E2[:, :])
        W2 = smt("W2")
        V.tensor_add(W2[:, :], u[:, :], Bfin[:, :])
        ctot = smt("ctot")
        V.tensor_add(ctot[:, :], m[:, :], W2[:, :])

        out_dma_holder.append(nc.sync.dma_start(out_h[:, :], ctot[:, :]))

    nc.finalize()
    return nc


def _get_nc() -> bass.Bass:
    global _NC_CACHE
    if _NC_CACHE is None:
        _NC_CACHE = _build_nc()
    return _NC_CACHE


def kernel(raw_sizes, raw_delays, raw_directions, delay_ms, padding_norm,
           confidence, profile_ids, trace=False, tmpdir=None):
    global LAST_RESULTS
    raw_sizes = np.asarray(raw_sizes, dtype=np.int32)
    raw_delays = np.asarray(raw_delays, dtype=np.float32)
    raw_directions = np.asarray(raw_directions, dtype=np.int32)
    delay_ms = np.asarray(delay_ms, dtype=np.float32)
    padding_norm = np.asarray(padding_norm, dtype=np.float32)
    confidence = np.asarray(confidence, dtype=np.float32)
    profile_ids = np.asarray(profile_ids).astype(np.int32)

    nc = _get_nc()
    in_maps = []
    for i in range(N_CORES):
        r = slice(i * BC, (i + 1) * BC)
        in_maps.append({
            "raw_sizes": raw_sizes[r],
            "raw_delays": raw_delays[r],
            "raw_directions": raw_directions[r],
            "delay_ms": delay_ms[r],
            "padding_norm": padding_norm[r],
            "confidence": confidence[r],
            "profile_ids": profile_ids[r],
        })

    LAST_RESULTS = run_bass_kernel_spmd(nc, in_maps, list(range(N_CORES)),
                                        trace=trace, tmpdir=tmpdir)
    partials = [LAST_RESULTS.results[i]["partial"] for i in range(N_CORES)]
    total = float(np.sum(np.stack(partials), dtype=np.float64))
    return np.float32(total / B)
